# revision 2
# baseline (speedup 1.0000x reference)
"""AttentionRNNLayer kernel v3: data-parallel (batch 64 -> 8 cores x 8),
raw-bass hand-scheduled scan.

Per step (L ~= 2.9us target):
  T: 64 LDW+MM pairs, group order gc, f, i, o (4 k-chunks each x 4 m-chunks),
     psum preloaded with xz by V-side CASTs (per-gate-group banks).
  S: tanh_gc, sig_f, sig_i, sig_o, tanh_c  (5 ACTs, ~320ns each)
  V: t1 = si*tgc, t2 = sf*c, c = t1+t2, [CAST_gc', CAST_f'], h = so*tct,
     [CAST_i', CAST_o']   (CASTs preload next step's psum in V-queue slack)
Cross-engine sync via manual semaphores with cumulative thresholds; waits are
fused onto the first consuming instruction of each group.
"""

import numpy as np
import ml_dtypes

import concourse.bacc as bacc
import concourse.mybir as mybir
import concourse.tile as tile
from concourse import bass_utils

B, T, F, U = 64, 256, 64, 512
G = 4 * U
NCORES = 8
BL = B // NCORES     # 8 batch per core
NK = U // 128        # 4 contraction chunks
NM = G // 128        # 16 gate chunks
BF16 = mybir.dt.bfloat16
F32 = mybir.dt.float32
AF = mybir.ActivationFunctionType

# gate chunk layout after host permutation: i=0:4, f=4:8, o=8:12, gc=12:16
# burst group order: gc, f, i, o  (c-chain inputs early, o last)
GROUPS = [  # (tag, chunk_base, func)
    ("gc", 12, AF.Tanh),
    ("f", 4, AF.Sigmoid),
    ("i", 0, AF.Sigmoid),
    ("o", 8, AF.Sigmoid),
]

LAST_RESULTS = None


def build_nc(t_steps=T):
    import sys, time
    print(f"[kernel3] build_nc(t={t_steps}) start", file=sys.stderr, flush=True)
    _t0 = time.time()
    nc = bacc.Bacc("TRN2", target_bir_lowering=False, debug=False)
    TS = t_steps

    d_xT = nc.dram_tensor("xT", [F + 1, TS * BL], BF16, kind="ExternalInput")
    d_kaug = nc.dram_tensor("kaug", [F + 1, G], BF16, kind="ExternalInput")
    d_w = nc.dram_tensor("w", [128, NK, G], BF16, kind="ExternalInput")
    d_outw = nc.dram_tensor("outw", [128, NK], BF16, kind="ExternalInput")
    d_out = nc.dram_tensor("out", [1, TS * BL], F32, kind="ExternalOutput")

    xT = nc.alloc_sbuf_tensor("xT_sb", [F + 1, TS * BL], BF16)
    kaug = nc.alloc_sbuf_tensor("kaug_sb", [F + 1, G], BF16)
    w = nc.alloc_sbuf_tensor("w_sb", [128, NK, NM, 128], BF16)
    outw = nc.alloc_sbuf_tensor("outw_sb", [128, NK], BF16)
    xz = nc.alloc_sbuf_tensor("xz_sb", [128, NM, TS, BL], BF16)
    hh = nc.alloc_sbuf_tensor("hh_sb", [128, TS, NK, BL], BF16)
    act = nc.alloc_sbuf_tensor("act_sb", [128, NM, BL], F32)
    tct = nc.alloc_sbuf_tensor("tct_sb", [128, NK, BL], F32)
    t1b = nc.alloc_sbuf_tensor("t1_sb", [128, NK, BL], F32)
    t2b = nc.alloc_sbuf_tensor("t2_sb", [128, NK, BL], F32)
    c_sb = nc.alloc_sbuf_tensor("c_sb", [128, NK, BL], F32)
    po = nc.alloc_sbuf_tensor("po_sb", [1, TS * BL], F32)

    _cleanup = nc.cleanup_on_exit()
    _cleanup.__enter__()

    # one psum bank per gate group (single-buffered; timing separates steps)
    ps = {
        tag: nc.alloc_psum_tensor(f"ps_{tag}", [128, NK, BL], F32)
        for tag, _, _ in GROUPS
    }
    psx2 = None

    mmdone = nc.alloc_semaphore("mmdone")   # +4/step (one per group)
    xzmm = nc.alloc_semaphore("xzmm")       # interleaved xz MMs
    xzcp = nc.alloc_semaphore("xzcp")       # interleaved xz copies
    outmm = nc.alloc_semaphore("outmm")     # interleaved output-dot MMs
    gdone = nc.alloc_semaphore("gdone")     # +1/step (t2 on gpsimd)
    sdone = nc.alloc_semaphore("sdone")     # +5/step
    vdone = nc.alloc_semaphore("vdone")     # +2/step (c, h)
    pre = nc.alloc_semaphore("pre")         # +4/step (CAST preloads)

    wblk = min(512, TS * BL)
    nblk = (TS * BL) // wblk
    tpb = wblk // BL

    # ---- (loads + xz-block-0 precompute moved into the raw scan block) ----
    njobs = NM * (nblk - 1)
    psx2 = [nc.alloc_psum_tensor(f"psx2_{p}", [128, wblk], F32) for p in range(2)]
    dsem_a = nc.alloc_semaphore("dsem_a")   # xT + kaug
    dsem_b = nc.alloc_semaphore("dsem_b")   # w + outw

    out_steps = {tpb * (b2 + 1) + 4: b2 for b2 in range(nblk - 1)} if njobs else {}

    # ---- raw scan (includes load + xz prologue) ----
    with nc.Block("scan") as blk:

        @blk.sync
        def _(sy):
            sy.dma_start(out=xT[:], in_=d_xT.ap()).then_inc(dsem_a, 16)
            sy.dma_start(out=kaug[:], in_=d_kaug.ap()).then_inc(dsem_a, 16)
            for k in range(NK):
                sy.dma_start(out=w[:, k, :, :], in_=d_w.ap()[:, k, :]).then_inc(
                    dsem_b, 16
                )
            sy.dma_start(out=outw[:], in_=d_outw.ap()).then_inc(dsem_b, 16)

        @blk.gpsimd
        def _(g):
            for t in range(TS):
                g.tensor_mul(
                    t2b[:], act[:, 4:8, :], c_sb[:]
                )._wait_ge(sdone, 5 * t + 2).then_inc(gdone)

        @blk.tensor
        def _(te):
            # standalone wait: the first matmul's LDWEIGHTS (reading kaug)
            # executes before any wait fused onto the MATMUL itself
            te.wait_ge(dsem_a, 32)  # xT + kaug loaded
            for m in range(NM):
                ins = te.matmul(
                    psx2[m % 2][:],
                    lhsT=kaug[:, 128 * m : 128 * (m + 1)],
                    rhs=xT[:, 0:wblk],
                    start=True,
                    stop=True,
                )
                if m >= 2:
                    ins._wait_ge(xzcp, m - 1)
                ins.then_inc(xzmm)
            te.wait_ge(dsem_b, 16 * (NK + 1))  # w + outw loaded
            for t in range(1, TS):
                if t in out_steps:
                    b2 = out_steps[t]
                    for k in range(NK):
                        ins = te.matmul(
                            psx2[0][0:1, :],
                            lhsT=outw[:, k : k + 1],
                            rhs=hh[:, tpb * b2 : tpb * (b2 + 1), k, :],
                            start=(k == 0),
                            stop=(k == NK - 1),
                        )
                        if k == NK - 1:
                            ins.then_inc(outmm)
                j = t - 1
                if j < njobs:
                    b = 1 + j // NM
                    m = j % NM
                    ins = te.matmul(
                        psx2[j % 2][:],
                        lhsT=kaug[:, 128 * m : 128 * (m + 1)],
                        rhs=xT[:, wblk * b : wblk * (b + 1)],
                        start=True,
                        stop=True,
                    )
                    ins._wait_ge(xzcp, NM + j - 1)
                    ins.then_inc(xzmm)
                for g, (tag, base, _) in enumerate(GROUPS):
                    for mi in range(4):
                        m = base + mi
                        for k in range(NK):
                            ins = te.matmul(
                                ps[tag][:, mi, :],
                                lhsT=w[:, k, m, :],
                                rhs=hh[:, t - 1, k, :],
                                start=False,
                                stop=(k == NK - 1),
                                skip_group_check=True,
                            )
                            if mi == 0 and k == 0:
                                if g == 0:
                                    # h_{t-1} ready; also covers CAST_gc/f(t)
                                    # (they precede h(t-1) in V's FIFO)
                                    ins._wait_ge(vdone, 2 * t)
                                elif g >= 2:
                                    # CAST_i/o(t) emitted after h(t-1)
                                    ins._wait_ge(pre, 4 * t + g + 1)
                            if mi == 3 and k == NK - 1:
                                ins.then_inc(mmdone)

        @blk.scalar
        def _(s):
            for t in range(TS):
                md = 4 * (t - 1)
                for g, (tag, base, func) in enumerate(GROUPS):
                    ins = s.activation(act[:, base : base + 4, :], ps[tag][:], func)
                    if t == 0:
                        ins._wait_ge(pre, g + 1)
                    else:
                        ins._wait_ge(mmdone, md + g + 1)
                    ins.then_inc(sdone)
                s.activation(
                    tct[:], c_sb[:], AF.Tanh
                )._wait_ge(vdone, 2 * t + 1).then_inc(sdone)

        @blk.vector
        def _(v):
            v.memset(c_sb[:], 0.0)
            for m in range(NM):
                v.tensor_copy(
                    out=xz[:, m, 0:tpb, :], in_=psx2[m % 2][:]
                )._wait_ge(xzmm, m + 1).then_inc(xzcp)
            # prologue: preload step-0 psum from xz[0]
            for g, (tag, base, _) in enumerate(GROUPS):
                ins = v.tensor_copy(out=ps[tag][:], in_=xz[:, base : base + 4, 0, :])
                ins.then_inc(pre)
            for t in range(TS):
                sd = 5 * t
                v.tensor_mul(
                    t1b[:], act[:, 0:4, :], act[:, 12:16, :]
                )._wait_ge(sdone, sd + 3)
                v.tensor_add(
                    c_sb[:], t1b[:], t2b[:]
                )._wait_ge(gdone, t + 1).then_inc(vdone)
                if t + 1 < TS:
                    for g, (tag, base, _) in enumerate(GROUPS[:2]):  # gc, f
                        ins = v.tensor_copy(
                            out=ps[tag][:], in_=xz[:, base : base + 4, t + 1, :]
                        )
                        if g == 0 and njobs and (t + 1) % tpb == 0:
                            ins._wait_ge(xzcp, NM * ((t + 1) // tpb - 1) + 2 * NM)
                        ins.then_inc(pre)
                v.tensor_mul(
                    hh[:, t, :, :], act[:, 8:12, :], tct[:]
                )._wait_ge(sdone, sd + 5).then_inc(vdone)
                if t + 1 < TS:
                    for g, (tag, base, _) in enumerate(GROUPS[2:]):  # i, o
                        v.tensor_copy(
                            out=ps[tag][:], in_=xz[:, base : base + 4, t + 1, :]
                        ).then_inc(pre)
                j = t - 1
                if 0 <= j < njobs:
                    b = 1 + j // NM
                    m = j % NM
                    v.tensor_copy(
                        out=xz[:, m, tpb * b : tpb * (b + 1), :], in_=psx2[j % 2][:]
                    )._wait_ge(xzmm, NM + j + 1).then_inc(xzcp)
                if t - 1 in out_steps:
                    b2 = out_steps[t - 1]
                    v.tensor_copy(
                        out=po[0:1, wblk * b2 : wblk * (b2 + 1)],
                        in_=psx2[0][0:1, :],
                    )._wait_ge(outmm, b2 + 1)

    # ---- output stage ----
    blk2_start = (nblk - 1) if njobs else 0
    with tile.TileContext(nc) as tc:
        with tc.tile_pool(name="psd", bufs=2, space="PSUM") as psd:
            for blk2 in range(blk2_start, nblk):
                pd = psd.tile([128, wblk], F32, tag="pd")
                for k in range(NK):
                    nc.tensor.matmul(
                        pd[0:1, :],
                        lhsT=outw[:, k : k + 1],
                        rhs=hh[:, tpb * blk2 : tpb * (blk2 + 1), k, :],
                        start=(k == 0),
                        stop=(k == NK - 1),
                    )
                nc.any.tensor_copy(
                    out=po[0:1, wblk * blk2 : wblk * (blk2 + 1)], in_=pd[0:1, :]
                )
            nc.sync.dma_start(out=d_out.ap(), in_=po[:])

    _cleanup.__exit__(None, None, None)
    print(f"[kernel3] tile done {time.time() - _t0:.1f}s; compiling...",
          file=sys.stderr, flush=True)
    nc.compile()
    print(f"[kernel3] build done {time.time() - _t0:.1f}s", file=sys.stderr,
          flush=True)
    return nc


_NC_CACHE = {}


def _get_nc(t_steps):
    if t_steps not in _NC_CACHE:
        _NC_CACHE[t_steps] = build_nc(t_steps)
    return _NC_CACHE[t_steps]


def _prep(inputs, kernel, rec_kernel, bias, att_W, att_b, out_W, out_b, t_steps):
    bf = ml_dtypes.bfloat16
    TS = t_steps
    # gate permutation: [i, f, o, gc]
    perm = np.concatenate(
        [np.arange(0, U), np.arange(U, 2 * U), np.arange(3 * U, 4 * U),
         np.arange(2 * U, 3 * U)]
    )
    kp = kernel[:, perm].astype(np.float32)
    bp = bias[perm].astype(np.float32)
    wp = rec_kernel[:, perm].astype(np.float32)

    kaug = np.concatenate([kp, bp[None, :]], axis=0).astype(bf)
    wt = np.ascontiguousarray(wp.reshape(NK, 128, G).transpose(1, 0, 2)).astype(bf)
    outw = np.ascontiguousarray(
        out_W.astype(np.float32).reshape(NK, 128).T
    ).astype(bf)

    xTs = []
    for ci in range(NCORES):
        xc = inputs[ci * BL : (ci + 1) * BL, :TS, :].astype(np.float32)
        xt = np.ascontiguousarray(xc.transpose(2, 1, 0).reshape(F, TS * BL))
        xt = np.concatenate([xt, np.ones((1, TS * BL), np.float32)], axis=0)
        xTs.append(xt.astype(bf))

    t_idx = np.arange(TS)
    tri = (t_idx[None, :] < t_idx[:, None]).astype(np.float32)
    aW = att_W.astype(np.float32)[:TS, :TS]
    coef = np.where(t_idx == 0, 1.0, (aW * tri).sum(axis=1))
    shift = np.where(t_idx == 0, 0.0, att_b.astype(np.float32)[:TS])
    shift2 = shift * float(out_W.astype(np.float32).sum()) + float(out_b)

    shared = {"kaug": kaug, "w": wt, "outw": outw}
    return shared, xTs, coef, shift2


def kernel(inputs, kernel, rec_kernel, bias, att_W, att_b, out_W, out_b,
           _t_steps=None):
    global LAST_RESULTS
    t_steps = _t_steps or T
    nc = _get_nc(t_steps)
    shared, xTs, coef, shift2 = _prep(
        np.asarray(inputs), np.asarray(kernel), np.asarray(rec_kernel),
        np.asarray(bias), np.asarray(att_W), np.asarray(att_b),
        np.asarray(out_W), np.asarray(out_b), t_steps,
    )
    in_maps = [{**shared, "xT": xTs[ci]} for ci in range(NCORES)]
    import sys, time
    print("[kernel3] launching spmd run...", file=sys.stderr, flush=True)
    _t0 = time.time()
    res = bass_utils.run_bass_kernel_spmd(nc, in_maps, core_ids=list(range(NCORES)))
    print(f"[kernel3] spmd run returned {time.time() - _t0:.1f}s",
          file=sys.stderr, flush=True)
    LAST_RESULTS = res

    TS = t_steps
    out = np.zeros((B, TS), np.float32)
    for ci in range(NCORES):
        dot = res.results[ci]["out"].astype(np.float64).reshape(TS, BL)  # [t, b]
        val = coef[:, None] * dot + shift2[:, None]
        sig = 1.0 / (1.0 + np.exp(-val))
        out[ci * BL : (ci + 1) * BL] = sig.T.astype(np.float32)
    return out
